# revision 48
# baseline (speedup 1.0000x reference)
"""CombinePatches (3D col2im fold + overlap-count normalize) on 8 TRN2 NeuronCores.

Decomposition (validated numerically against the reference):
  out[b, 2q+kd, 2s+kh, 2u+kw, c] (+)= patches[b, q, s, u, kd, kh, kw, c], then
  out /= cnt, cnt = cd(d)*ch(h)*cw(w) separable overlap counts.

Sharding: 8 cores = B(2) x D-chunks(4). Each core computes 16 output d-rows from
9 od-slices of patches (1 halo slice, zero-padded at global edges by the host).

Per core, per output row d (r=d%2, q=d//2):
  - DVE w-fold: T[s, j, w, c] = A[s, floor(w/2), j, ...] + A[s, floor(w/2)-1, ...]
    done for A = slice q (kd=r) and B = slice q-1 (kd=r+2), with the ow dim
    pre-split into two halves on partitions (p = uhalf*64 + s) so each DVE op
    uses all 128 lanes.
  - TensorE h-fold: O[h, (w,c)] = sum_j Mh_j^T @ T_j accumulated in PSUM over
    (j x {A,B} x {w-half}) = 16 float32r matmuls; 0.25*rh(h) baked into Mh
    (0.25 = interior rd * interior rw).
  - ScalarE eviction: PSUM -> SBUF copy, then DMA store on the scalar ring.
Host fixes the global d-edge rows and w-edge columns by x2 after gather.
"""
import sys

for _p in ("/opt/trn_rl_repo", "/opt/trn_rl_repo/pypackages"):
    if _p not in sys.path:
        sys.path.insert(0, _p)

from contextlib import ExitStack

import numpy as np

import concourse.bass as bass
import concourse.tile as tile
from concourse import bacc, mybir
from concourse import bass_utils

B, D, H, W, C = 2, 64, 128, 128, 4
od, oh, ow = 31, 63, 63
NS = 9              # od-slices per core (incl 1 halo)
RPC = 16            # output d-rows per core
MM_DT = mybir.dt.bfloat16
import ml_dtypes

BF16 = ml_dtypes.bfloat16

# per-partition free width of a slice with nkd kd-planes:
# [vpair=2][kd=nkd][j=4][x=32][t=2][c=4] -- vpair outermost, so the whole
# kw-fold of a slice is ONE fully contiguous DVE add (vp0 half + vp1 half),
# and each folded kd-plane is a contiguous j-major 1024-elem matmul rhs.
def _fw(nkd):
    return 2 * nkd * 1024


# 126 data partitions: p<63 = (uhalf 0, s=p), 63<=p<126 = (uhalf 1, s=p-63).
# The two all-zero s=63 pad rows are never transferred; they would land on
# partitions served by SDMA engine 15, which is ~20% slower than the rest
# and was the straggler that set the load-stream critical path. Matmuls
# run K=126 so the never-written partitions 126/127 are never read.
NP = 126
FULL2, HALF2 = _fw(4), _fw(2)   # DRAM elems/partition per slice
PP_TOTAL = NP * (2 * HALF2 + 7 * FULL2)

_cache = {}


def _build():
    nc = bacc.Bacc(
        "TRN2",
        target_bir_lowering=False,
        debug=False,
        enable_asserts=False,
        num_devices=8,
    )
    # flat pp: [half-slice k=0 (kd 2,3 only)] + [7 full slices] + [half k=8 (kd 0,1)]
    pp_d = nc.dram_tensor(
        "pp", [PP_TOTAL], MM_DT, kind="ExternalInput"
    ).ap()
    wm_d = nc.dram_tensor("wm", [NP, 1024], MM_DT, kind="ExternalInput").ap()
    out_d = nc.dram_tensor(
        "out", [RPC, H, W, C], MM_DT, kind="ExternalOutput"
    ).ap()

    with ExitStack() as ctx:
        tc = ctx.enter_context(tile.TileContext(nc))
        const_pool = ctx.enter_context(tc.tile_pool(name="const", bufs=1))
        # staged slice tiles have exactly one reader (the mega-fold), so
        # slots recycle immediately and a few bufs keep the DMA stream fed
        # without piling up outstanding DMAs (9 outstanding loads measurably
        # degraded early HBM throughput).
        slice_pool = ctx.enter_context(tc.tile_pool(name="slice", bufs=4))
        f_pool = ctx.enter_context(tc.tile_pool(name="fold", bufs=3))
        t_pool = ctx.enter_context(tc.tile_pool(name="tt", bufs=6))
        # every eviction gets its own buffer so all stores can be deferred
        # past the end of the load stream: store DMA packets otherwise
        # interleave with loads on the same SDMA engines and stretch the
        # load-stream critical path by ~5us.
        ev_pool = ctx.enter_context(tc.tile_pool(name="ev", bufs=8))
        psum_pool = ctx.enter_context(tc.tile_pool(name="ps", bufs=4, space="PSUM"))

        # constants go on the scalar-engine HWDGE ring so the sync ring is
        # purely slice loads (HWDGE rings are FIFO per issuing engine).
        wm_sb = const_pool.tile([NP, 1024], MM_DT)
        nc.scalar.dma_start(wm_sb[:], wm_d[:])

        def slice_region(k):
            """(flat offset, free width, n_kd, kd_base) of slice k."""
            if k == 0:
                return 0, HALF2, 2, 2
            if k == NS - 1:
                return NP * (HALF2 + 7 * FULL2), HALF2, 2, 0
            return NP * (HALF2 + (k - 1) * FULL2), FULL2, 4, 0

        folds = {}
        evs = []
        for k in range(NS):
            off, fw, nkd, kdb = slice_region(k)
            t = slice_pool.tile([NP, fw], MM_DT, tag="slice")
            # the DMA splitter uses the largest divisor of the partition
            # count <= 16 engines: 126 rows would spread over only 14. So
            # load 112 rows (16x7) on the sync ring and the remaining 14
            # rows (14x1) on the scalar ring.
            nc.sync.dma_start(
                t[0:112, :],
                pp_d[off : off + 112 * fw].rearrange("(p f) -> p f", f=fw),
            )
            nc.scalar.dma_start(
                t[112:NP, :],
                pp_d[off + 112 * fw : off + NP * fw].rearrange(
                    "(p f) -> p f", f=fw
                ),
            )
            # whole-slice kw-fold in ONE fully contiguous DVE add: the vp0
            # half plus the vp1 half. Contiguity keeps DVE SBUF-port traffic
            # minimal (strided 8-elem runs waste half of every 32B line and
            # that bank pressure slows DMA/PE under load), and one reader
            # frees the staged tile immediately for the next load.
            last = k == NS - 1
            if not last:
                F = f_pool.tile([NP, nkd * 1024], MM_DT, tag="F")
                nc.vector.tensor_add(
                    F[:], t[:, 0 : nkd * 1024], t[:, nkd * 1024 : 2 * nkd * 1024]
                )
                folds[k] = (F, kdb)
            if k == 0:
                continue
            Fb, b_kdb = folds[k - 1]
            bo = (2 - b_kdb) * 1024  # offset of kd=2 plane in Fb
            # kd-fold for BOTH rows in ONE contiguous 2048-elem DVE add:
            # Fa's kd0,kd1 planes are adjacent, as are Fb's kd2,kd3 planes,
            # so T2 = [T(row0) | T(row1)] directly (j-major rhs layout).
            T2 = t_pool.tile([NP, 2048], MM_DT, tag="T")
            if last:
                # final slice: kw-fold straight into a temp, then combine --
                # shortest DVE chain after the final load lands.
                tmp = t_pool.tile([NP, 2048], MM_DT, tag="T")
                nc.vector.tensor_add(tmp[:], t[:, 0:2048], t[:, 2048:4096])
                nc.vector.tensor_add(T2[:], tmp[:], Fb[:, bo : bo + 2048])
            else:
                Fa, a_kdb = folds[k]
                ao = (0 - a_kdb) * 1024
                nc.vector.tensor_add(
                    T2[:], Fa[:, ao : ao + 2048], Fb[:, bo : bo + 2048]
                )
            ev = ev_pool.tile([128, 1024], MM_DT, tag="ev")
            ps = psum_pool.tile([128, 1024], mybir.dt.float32, tag="ps")
            for rr in range(2):
                for half in range(2):
                    outseg = ps[:, rr * 512 + half * 256 : rr * 512 + (half + 1) * 256]
                    for j in range(4):
                        # K=126 (both zero s=63 pad rows dropped from the
                        # transfer); single PE tile position (0,0) as before.
                        lhsT = wm_sb[:, 512 * half + j * 128 : 512 * half + (j + 1) * 128]
                        rhs = T2[:, rr * 1024 + j * 256 : rr * 1024 + (j + 1) * 256]
                        nc.tensor.matmul(
                            outseg, lhsT, rhs, start=(j == 0), stop=(j == 3)
                        )
            # evict on ScalarE: evictions wait on matmuls, and in the DVE
            # FIFO they would delay later folds. rw's interior 0.5 is
            # folded into wm; host rescales the w edges.
            nc.scalar.copy(ev[:], ps[:])
            # stores interleave with loads on the scalar ring; deferring
            # them all past the load stream was measured slower (the tail
            # store drain costs more than the interleave steals).
            d0 = 2 * (k - 1)
            nc.scalar.dma_start(
                out_d[d0 : d0 + 2].rearrange("d h w c -> h d (w c)"),
                ev[:].rearrange("p (d f) -> p d f", d=2),
            )
    nc.compile()
    return nc


def _host_tables():
    rh = np.where(
        (np.arange(H) < 2) | (np.arange(H) >= H - 2), 1.0, 0.5
    ).astype(np.float32)
    # [uhalf*63+s, whalf*512 + j*128 + h], block-diagonal in (uhalf, whalf).
    # 0.25 = interior rd (0.5) * interior rw (0.5); host rescales d/w edges.
    wm = np.zeros((NP, 1024), np.float32)
    s_idx = np.arange(oh)
    for j in range(4):
        h = 2 * s_idx + j
        wm[s_idx, j * 128 + h] = 0.25 * rh[h]
        wm[63 + s_idx, 512 + j * 128 + h] = 0.25 * rh[h]
    return wm.astype(BF16)


def _shard_inputs(patches):
    """Build per-core flat patch blocks. Per slice the layout is
    [p=(uhalf,s)][vpair][kd][j][x=32][t][c] where vpair 0 = kw{0,1} at
    u-slots 1:33 and vpair 1 = kw{2,3} at u-slots 0:32; the two vpair
    halves are contiguous operands of one whole-slice kw-fold add, and
    each folded kd-plane is a contiguous j-major matmul rhs."""
    P5 = np.ascontiguousarray(patches).reshape(B, od, oh, ow, 256).astype(BF16)
    # q-slot k = q+1 for q in [-1, 32); u-slot x = u+1 for u in [-1, 65)
    Pu = np.zeros((B, od + 2, 64, 66, 4, 4, 4, 4), BF16)
    Pu.reshape(B, od + 2, 64, 66, 256)[:, 1 : od + 1, 0:oh, 1 : ow + 1, :] = P5
    pps = []
    for core in range(8):
        b, kc = core // 4, core % 4
        s0 = 8 * kc  # = qbase + 1
        parts = []
        for k in range(NS):
            if k == 0:
                nkd, kdb = 2, 2
            elif k == NS - 1:
                nkd, kdb = 2, 0
            else:
                nkd, kdb = 4, 0
            Q = Pu[b, s0 + k]  # [s=64, u=66, kd, j, v, c]
            blk = np.empty((2, 63, 2, nkd, 4, 32, 2, 4), BF16)
            for uh in range(2):
                for vp in range(2):
                    us = 32 * uh + (1 - vp)
                    # [s, x, kd, j, t, c] -> [s, kd, j, x, t, c]; drop the
                    # all-zero s=63 pad row (partitions are 2*63=126 wide).
                    blk[uh, :, vp] = np.transpose(
                        Q[:63, us : us + 32, kdb : kdb + nkd, :, 2 * vp : 2 * vp + 2, :],
                        (0, 2, 3, 1, 4, 5),
                    )
            parts.append(blk.reshape(-1))
        pps.append(np.concatenate(parts))
    return pps


def _run(patches, trace=False):
    if "nc" not in _cache:
        _cache["nc"] = _build()
        _cache["tables"] = _host_tables()
    nc = _cache["nc"]
    wm = _cache["tables"]
    pps = _shard_inputs(np.asarray(patches, dtype=np.float32))
    in_maps = [{"pp": pps[core], "wm": wm} for core in range(8)]
    res = bass_utils.run_bass_kernel_spmd(
        nc, in_maps, core_ids=list(range(8)), trace=trace
    )
    out = np.zeros((B, D, H, W, C), np.float32)
    for core in range(8):
        b, kc = core // 4, core % 4
        out[b, RPC * kc : RPC * (kc + 1)] = np.asarray(
            res.results[core]["out"]
        ).astype(np.float32)
    out[:, [0, 1, D - 2, D - 1]] *= 2.0
    out[:, :, :, [0, 1, W - 2, W - 1], :] *= 2.0
    return out, res


def kernel(patches, inputs):
    out, _ = _run(patches)
    return out

